# revision 25
# baseline (speedup 1.0000x reference)
"""Trainium2 Bass kernel for RNN classification forward pass.

Math (per reference):
    Wx = einsum('tbi,ih->tbh', X, W_enc) + b_enc          # encoder GEMM
    h_t = tanh(Wx_t + h_{t-1} @ W_h + b_h)                # T sequential steps
    return h_T                                            # [B, D_H]

Strategy: pure data-parallel over batch (8 cores x B_loc=16), no cross-core
communication.  Per-step collectives have a ~5-10us floor on trn2, which is
a non-starter for 512 sequential steps; and since a matmul's PE time is set
by the moving-operand stream (K/128 x N cycles, independent of M), the scan
costs the same ~8192 cyc/step per core no matter how the batch is sharded --
so data-parallel is as good as any partitioning that avoids communication.

Per core:
  Phase 1 (encoder): WxB[t,b,:] = X_shard @ W_enc + (b_enc + b_h) as one
    large GEMM over T*B_loc rows (full-128 stationary M), with PE-transposed
    X tiles as the stationary operand and fp32r matmuls (1 cyc/row at
    N>=256; plain fp32 would be 4 cyc/row).  The bias row is replicated to
    all 128 partitions once via a one-hot-stationary matmul and added in the
    DVE epilogue.  Result staged to a DRAM scratch buffer.
  Phase 2 (scan): state kept transposed (hT: 8 chunks of [128, 16]) so each
    step's matmuls use hT as the *stationary* operand (tiny 16-col weight
    loads) and stream W_h columns: psum[16,256] += hT_k.T @ W_h[k, qslice],
    4 psum quarters per step.  DVE adds WxB_t in place on PSUM, ACT applies
    tanh to SBUF, and PE transpose-mode ops (single-pass in fp32r) rebuild
    the hT chunks, two chunks packed per [128,32] psum tile with one cast.
    Transposes are scheduled where their semaphore waits are pre-satisfied:
    chunks 0-3 after this step's q3 matmuls, chunks 4-7 deferred into the
    NEXT step's q0 k-loop (at k=4), hiding the add+tanh latency behind
    matmul streams.

Measured on trn2 (8 cores): ~2.72 ms HW exec, rel err ~2.2e-4 vs fp32 ref.
"""
import numpy as np

import concourse.bass as bass
import concourse.mybir as mybir
import concourse.tile as tile
from concourse import bacc
from concourse.bass_utils import run_bass_kernel_spmd
from concourse.masks import make_identity

T, B, D_IN, D_H = 512, 128, 512, 1024
N_CORES = 8
B_LOC = B // N_CORES  # 16

F32 = mybir.dt.float32
F32R = mybir.dt.float32r
Tanh = mybir.ActivationFunctionType.Tanh

_BUILT = {}


def build(t_steps: int = T):
    """Build the per-core Bass module (SPMD; same program on all cores)."""
    nc = bacc.Bacc("TRN2", target_bir_lowering=False, debug=False)

    X = nc.dram_tensor("X", [t_steps, B_LOC, D_IN], F32, kind="ExternalInput").ap()
    H0 = nc.dram_tensor("h", [B_LOC, D_H], F32, kind="ExternalInput").ap()
    WENC = nc.dram_tensor("W_enc", [D_IN, D_H], F32, kind="ExternalInput").ap()
    BENC = nc.dram_tensor("b_enc", [D_H], F32, kind="ExternalInput").ap()
    WH = nc.dram_tensor("W_h", [D_H, D_H], F32, kind="ExternalInput").ap()
    BH = nc.dram_tensor("b_h", [D_H], F32, kind="ExternalInput").ap()
    OUT = nc.dram_tensor("out", [B_LOC, D_H], F32, kind="ExternalOutput").ap()
    WXB = nc.dram_tensor("wxb_scratch", [t_steps * B_LOC, D_H], F32).ap()

    KI = D_IN // 128  # 4 K-chunks for the encoder contraction
    KH = D_H // 128   # 8 K-chunks for the recurrent contraction
    NSEG = D_H // 256  # 4 scan output segments (psum tiles of [16, 256])
    ROWS = t_steps * B_LOC
    assert ROWS % 128 == 0
    NCH = ROWS // 128  # (t,b)-row chunks for the encoder

    with tile.TileContext(nc) as tc:
        with (
            tc.tile_pool(name="const", bufs=1) as const_pool,
        ):
            # ---- constants ----
            ident = const_pool.tile([128, 128], F32)
            make_identity(nc, ident[:])
            identr = const_pool.tile([128, 128], F32R)
            nc.vector.tensor_copy(identr[:], ident[:])

            # e0: ones in partitions 0-1 -> bias matmul adds bvec row0+row1.
            e0f = const_pool.tile([128, 128], F32)
            nc.gpsimd.memset(e0f[:], 0.0)
            nc.gpsimd.memset(e0f[0:2, :], 1.0)
            e0 = const_pool.tile([128, 128], F32R)
            nc.vector.tensor_copy(e0[:], e0f[:])

            # bvec: row 0 = b_enc, row 1 = b_h, other partitions zero.
            zeros_f = const_pool.tile([128, D_H], F32)
            nc.gpsimd.memset(zeros_f[:], 0.0)
            bvec = const_pool.tile([128, D_H], F32R)
            nc.vector.tensor_copy(bvec[:], zeros_f[:])
            nc.sync.dma_start(bvec[0:1, :], BENC[None, :].bitcast(F32R))
            nc.sync.dma_start(bvec[1:2, :], BH[None, :].bitcast(F32R))

            # resident weights, K-chunked with K on partitions
            wh_sb = const_pool.tile([128, KH, D_H], F32R)
            nc.sync.dma_start(
                wh_sb[:], WH.rearrange("(ko ki) n -> ki ko n", ki=128).bitcast(F32R)
            )
            wenc_sb = const_pool.tile([128, KI, D_H], F32R)
            nc.sync.dma_start(
                wenc_sb[:], WENC.rearrange("(ko ki) n -> ki ko n", ki=128).bitcast(F32R)
            )

            # bias_rep: every row = b_enc + b_h (one e0-matmul per 512 cols)
            bias_rep = const_pool.tile([128, D_H], F32)
            with tc.tile_pool(name="bias_ps", bufs=2, space="PSUM") as bias_ps:
                for n in range(D_H // 512):
                    nsl = bass.ts(n, 512)
                    bps = bias_ps.tile([128, 512], F32)
                    nc.tensor.matmul(bps[:], e0[:], bvec[:, nsl], start=True, stop=True)
                    nc.scalar.copy(bias_rep[:, nsl], bps[:])

            # ---- phase 1: encoder GEMM -> WXB scratch ----
            Xf = X.rearrange("t b i -> (t b) i")
            with (
                tc.tile_pool(name="enc_x", bufs=4) as x_pool,
                tc.tile_pool(name="enc_xt", bufs=3) as xt_pool,
                tc.tile_pool(name="enc_ps", bufs=3, space="PSUM") as enc_ps,
                tc.tile_pool(name="enc_tps", bufs=2, space="PSUM") as enc_tps,
                tc.tile_pool(name="enc_out", bufs=4) as enc_out,
            ):
                for c in range(NCH):
                    xnat = x_pool.tile([128, D_IN], F32R)
                    nc.sync.dma_start(
                        xnat[:], Xf[128 * c : 128 * (c + 1), :].bitcast(F32R)
                    )
                    xT = xt_pool.tile([128, KI, 128], F32R)
                    for k in range(KI):
                        tps = enc_tps.tile([128, 128], F32R)
                        nc.tensor.transpose(
                            tps[:], xnat[:, 128 * k : 128 * (k + 1)], identr[:]
                        )
                        nc.scalar.copy(xT[:, k, :], tps[:])
                    for n in range(D_H // 512):
                        nsl = bass.ts(n, 512)
                        ps = enc_ps.tile([128, 512], F32)
                        for k in range(KI):
                            nc.tensor.matmul(
                                ps[:],
                                xT[:, k, :],
                                wenc_sb[:, k, nsl],
                                start=(k == 0),
                                stop=(k == KI - 1),
                            )
                        wxbo = enc_out.tile([128, 512], F32)
                        nc.vector.tensor_add(wxbo[:], ps[:], bias_rep[:, nsl])
                        nc.sync.dma_start(WXB[128 * c : 128 * (c + 1), nsl], wxbo[:])

            # ---- phase 2: sequential scan ----
            with (
                tc.tile_pool(name="wxb", bufs=8) as wxb_pool,
                tc.tile_pool(name="hT", bufs=24) as hT_pool,
                tc.tile_pool(name="hseg", bufs=10) as hseg_pool,
                tc.tile_pool(name="scan_ps", bufs=5, space="PSUM") as scan_ps,
                tc.tile_pool(name="scan_tps", bufs=3, space="PSUM") as scan_tps,
                tc.tile_pool(name="h0p", bufs=1) as h0_pool,
            ):
                # initial state -> transposed chunks (packed 2 per tile)
                h0 = h0_pool.tile([B_LOC, D_H], F32R)
                nc.sync.dma_start(h0[:], H0[:, :].bitcast(F32R))
                hT = [None] * KH
                def pack_transposes(hseg_pair_src, c0, dest):
                    """Transpose two [B_LOC,128] column blocks of hseg_pair_src
                    into one [128, 2*B_LOC] psum tile; single cast to an f32r
                    hT tile holding chunks c0, c0+1."""
                    tps = scan_tps.tile([128, 2 * B_LOC], F32R)
                    for j in range(2):
                        nc.tensor.transpose(
                            tps[:, bass.ts(j, B_LOC)],
                            hseg_pair_src[:, 128 * j : 128 * (j + 1)],
                            identr[:B_LOC, :B_LOC],
                        )
                    ht2 = hT_pool.tile([128, 2 * B_LOC], F32R)
                    nc.vector.tensor_copy(ht2[:], tps[:])
                    dest[c0] = ht2[:, 0:B_LOC]
                    dest[c0 + 1] = ht2[:, B_LOC : 2 * B_LOC]


                for cpos in range(0, KH, 2):
                    pack_transposes(h0[:, 256 * (cpos // 2) :], cpos, hT)

                pending = None  # (hseg_q2, hseg_q3, dest) from previous step
                for t in range(t_steps):
                    wxb = wxb_pool.tile([B_LOC, D_H], F32)
                    nc.sync.dma_start(wxb[:], WXB[B_LOC * t : B_LOC * (t + 1), :])
                    hT_new = [None] * KH
                    hsegs = []
                    for q in range(4):
                        qsl = bass.ts(q, 256)
                        ps = scan_ps.tile([B_LOC, 256], F32)
                        for ki in range(KH):
                            if ki == 4 and pending is not None:
                                # previous step's chunks 4-7 (tanh completed
                                # during this step's q0 k0-k3 matmuls)
                                pack_transposes(pending[0], 4, pending[2])
                                pack_transposes(pending[1], 6, pending[2])
                                pending = None
                            nc.tensor.matmul(
                                ps[:],
                                hT[ki][:],
                                wh_sb[:, ki, qsl],
                                start=(ki == 0),
                                stop=(ki == KH - 1),
                            )
                        if q == 3 and t < t_steps - 1:
                            # chunks 0-3: q0/q1 tanh completed during q1-q3 MMs
                            pack_transposes(hsegs[0], 0, hT_new)
                            pack_transposes(hsegs[1], 2, hT_new)
                        nc.vector.tensor_add(ps[:], ps[:], wxb[:, qsl])
                        hseg = hseg_pool.tile([B_LOC, 256], F32R)
                        nc.scalar.activation(hseg[:], ps[:], Tanh)
                        if t == t_steps - 1:
                            nc.sync.dma_start(OUT[:, qsl].bitcast(F32R), hseg[:])
                        else:
                            hsegs.append(hseg)
                    if t < t_steps - 1:
                        pending = (hsegs[2], hsegs[3], hT_new)
                        hT = hT_new

    nc.compile()
    return nc


def _get(t_steps: int = T):
    if t_steps not in _BUILT:
        _BUILT[t_steps] = build(t_steps)
    return _BUILT[t_steps]


def run(inputs: dict, t_steps: int = T, trace: bool = False, **kw):
    """Shard inputs, run on 8 cores, gather. Returns (out, BassKernelResults)."""
    nc = _get(t_steps)
    X = np.ascontiguousarray(np.asarray(inputs["X"], dtype=np.float32))
    h = np.ascontiguousarray(np.asarray(inputs["h"], dtype=np.float32))
    W_enc = np.ascontiguousarray(np.asarray(inputs["W_enc"], dtype=np.float32))
    b_enc = np.ascontiguousarray(np.asarray(inputs["b_enc"], dtype=np.float32))
    W_h = np.ascontiguousarray(np.asarray(inputs["W_h"], dtype=np.float32))
    b_h = np.ascontiguousarray(np.asarray(inputs["b_h"], dtype=np.float32))
    assert X.shape == (t_steps, B, D_IN), X.shape

    in_maps = []
    for j in range(N_CORES):
        bs = slice(j * B_LOC, (j + 1) * B_LOC)
        in_maps.append(
            {
                "X": np.ascontiguousarray(X[:, bs, :]),
                "h": np.ascontiguousarray(h[bs, :]),
                "W_enc": W_enc,
                "b_enc": b_enc,
                "W_h": W_h,
                "b_h": b_h,
            }
        )
    res = run_bass_kernel_spmd(
        nc, in_maps, core_ids=list(range(N_CORES)), trace=trace, **kw
    )
    out = np.concatenate([r["out"] for r in res.results], axis=0)
    return out, res


def kernel(**inputs) -> np.ndarray:
    out, _ = run(inputs, T)
    return out


# revision 26
# speedup vs baseline: 1.0040x; 1.0040x over previous
"""Trainium2 Bass kernel for RNN classification forward pass.

Math (per reference):
    Wx = einsum('tbi,ih->tbh', X, W_enc) + b_enc          # encoder GEMM
    h_t = tanh(Wx_t + h_{t-1} @ W_h + b_h)                # T sequential steps
    return h_T                                            # [B, D_H]

Strategy: pure data-parallel over batch (8 cores x B_loc=16), no cross-core
communication.  Per-step collectives have a ~5-10us floor on trn2, which is
a non-starter for 512 sequential steps; and since a matmul's PE time is set
by the moving-operand stream (K/128 x N cycles, independent of M), the scan
costs the same ~8192 cyc/step per core no matter how the batch is sharded --
so data-parallel is as good as any partitioning that avoids communication.

Per core:
  Phase 1 (encoder): WxB[t,b,:] = X_shard @ W_enc + (b_enc + b_h) as one
    large GEMM over T*B_loc rows (full-128 stationary M), with PE-transposed
    X tiles as the stationary operand and fp32r matmuls (1 cyc/row at
    N>=256; plain fp32 would be 4 cyc/row).  The bias row is replicated to
    all 128 partitions once via a one-hot-stationary matmul and added in the
    DVE epilogue.  Result staged to a DRAM scratch buffer.
  Phase 2 (scan): state kept transposed (hT: 8 chunks of [128, 16]) so each
    step's matmuls use hT as the *stationary* operand (tiny 16-col weight
    loads) and stream W_h columns: psum[16,256] += hT_k.T @ W_h[k, qslice],
    4 psum quarters per step.  DVE adds WxB_t in place on PSUM, ACT applies
    tanh to SBUF, and PE transpose-mode ops (single-pass in fp32r) rebuild
    the hT chunks, two chunks packed per [128,32] psum tile with one cast.
    Transposes are scheduled where their semaphore waits are pre-satisfied:
    chunks 0-3 after this step's q3 matmuls, chunks 4-7 deferred into the
    NEXT step's q0 k-loop (at k=4), hiding the add+tanh latency behind
    matmul streams.

Measured on trn2 (8 cores): ~2.72 ms HW exec, rel err ~2.2e-4 vs fp32 ref.
"""
import numpy as np

import concourse.bass as bass
import concourse.mybir as mybir
import concourse.tile as tile
from concourse import bacc
from concourse.bass_utils import run_bass_kernel_spmd
from concourse.masks import make_identity

T, B, D_IN, D_H = 512, 128, 512, 1024
N_CORES = 8
B_LOC = B // N_CORES  # 16

F32 = mybir.dt.float32
F32R = mybir.dt.float32r
Tanh = mybir.ActivationFunctionType.Tanh

_BUILT = {}


def build(t_steps: int = T):
    """Build the per-core Bass module (SPMD; same program on all cores)."""
    nc = bacc.Bacc("TRN2", target_bir_lowering=False, debug=False)

    X = nc.dram_tensor("X", [t_steps, B_LOC, D_IN], F32, kind="ExternalInput").ap()
    H0 = nc.dram_tensor("h", [B_LOC, D_H], F32, kind="ExternalInput").ap()
    WENC = nc.dram_tensor("W_enc", [D_IN, D_H], F32, kind="ExternalInput").ap()
    BENC = nc.dram_tensor("b_enc", [D_H], F32, kind="ExternalInput").ap()
    WH = nc.dram_tensor("W_h", [D_H, D_H], F32, kind="ExternalInput").ap()
    BH = nc.dram_tensor("b_h", [D_H], F32, kind="ExternalInput").ap()
    OUT = nc.dram_tensor("out", [B_LOC, D_H], F32, kind="ExternalOutput").ap()
    WXB = nc.dram_tensor("wxb_scratch", [t_steps * B_LOC, D_H], F32).ap()

    KI = D_IN // 128  # 4 K-chunks for the encoder contraction
    KH = D_H // 128   # 8 K-chunks for the recurrent contraction
    NSEG = D_H // 256  # 4 scan output segments (psum tiles of [16, 256])
    ROWS = t_steps * B_LOC
    assert ROWS % 128 == 0
    NCH = ROWS // 128  # (t,b)-row chunks for the encoder

    with tile.TileContext(nc) as tc:
        with (
            tc.tile_pool(name="const", bufs=1) as const_pool,
        ):
            # ---- constants ----
            ident = const_pool.tile([128, 128], F32)
            make_identity(nc, ident[:])
            identr = const_pool.tile([128, 128], F32R)
            nc.vector.tensor_copy(identr[:], ident[:])

            # e0: ones in partitions 0-1 -> bias matmul adds bvec row0+row1.
            e0f = const_pool.tile([128, 128], F32)
            nc.gpsimd.memset(e0f[:], 0.0)
            nc.gpsimd.memset(e0f[0:2, :], 1.0)
            e0 = const_pool.tile([128, 128], F32R)
            nc.vector.tensor_copy(e0[:], e0f[:])

            # bvec: row 0 = b_enc, row 1 = b_h, other partitions zero.
            zeros_f = const_pool.tile([128, D_H], F32)
            nc.gpsimd.memset(zeros_f[:], 0.0)
            bvec = const_pool.tile([128, D_H], F32R)
            nc.vector.tensor_copy(bvec[:], zeros_f[:])
            nc.sync.dma_start(bvec[0:1, :], BENC[None, :].bitcast(F32R))
            nc.sync.dma_start(bvec[1:2, :], BH[None, :].bitcast(F32R))

            # resident weights, K-chunked with K on partitions
            wh_sb = const_pool.tile([128, KH, D_H], F32R)
            nc.sync.dma_start(
                wh_sb[:], WH.rearrange("(ko ki) n -> ki ko n", ki=128).bitcast(F32R)
            )
            wenc_sb = const_pool.tile([128, KI, D_H], F32R)
            nc.sync.dma_start(
                wenc_sb[:], WENC.rearrange("(ko ki) n -> ki ko n", ki=128).bitcast(F32R)
            )

            # bias_rep: every row = b_enc + b_h (one e0-matmul per 512 cols)
            bias_rep = const_pool.tile([128, D_H], F32)
            with tc.tile_pool(name="bias_ps", bufs=2, space="PSUM") as bias_ps:
                for n in range(D_H // 512):
                    nsl = bass.ts(n, 512)
                    bps = bias_ps.tile([128, 512], F32)
                    nc.tensor.matmul(bps[:], e0[:], bvec[:, nsl], start=True, stop=True)
                    nc.scalar.copy(bias_rep[:, nsl], bps[:])

            # ---- phase 1: encoder GEMM -> WXB scratch ----
            Xf = X.rearrange("t b i -> (t b) i")
            with (
                tc.tile_pool(name="enc_x", bufs=4) as x_pool,
                tc.tile_pool(name="enc_xt", bufs=3) as xt_pool,
                tc.tile_pool(name="enc_ps", bufs=3, space="PSUM") as enc_ps,
                tc.tile_pool(name="enc_tps", bufs=2, space="PSUM") as enc_tps,
                tc.tile_pool(name="enc_out", bufs=4) as enc_out,
            ):
                for c in range(NCH):
                    xnat = x_pool.tile([128, D_IN], F32R)
                    nc.sync.dma_start(
                        xnat[:], Xf[128 * c : 128 * (c + 1), :].bitcast(F32R)
                    )
                    xT = xt_pool.tile([128, KI, 128], F32R)
                    for k in range(KI):
                        tps = enc_tps.tile([128, 128], F32R)
                        nc.tensor.transpose(
                            tps[:], xnat[:, 128 * k : 128 * (k + 1)], identr[:]
                        )
                        nc.scalar.copy(xT[:, k, :], tps[:])
                    for n in range(D_H // 512):
                        nsl = bass.ts(n, 512)
                        ps = enc_ps.tile([128, 512], F32)
                        for k in range(KI):
                            nc.tensor.matmul(
                                ps[:],
                                xT[:, k, :],
                                wenc_sb[:, k, nsl],
                                start=(k == 0),
                                stop=(k == KI - 1),
                            )
                        wxbo = enc_out.tile([128, 512], F32)
                        nc.vector.tensor_add(wxbo[:], ps[:], bias_rep[:, nsl])
                        nc.sync.dma_start(WXB[128 * c : 128 * (c + 1), nsl], wxbo[:])

            # ---- phase 2: sequential scan ----
            with (
                tc.tile_pool(name="wxb", bufs=6) as wxb_pool,
                tc.tile_pool(name="hT", bufs=24) as hT_pool,
                tc.tile_pool(name="hseg", bufs=8) as hseg_pool,
                tc.tile_pool(name="scan_ps", bufs=5, space="PSUM") as scan_ps,
                tc.tile_pool(name="scan_tps", bufs=3, space="PSUM") as scan_tps,
                tc.tile_pool(name="h0p", bufs=1) as h0_pool,
            ):
                # initial state -> transposed chunks (packed 2 per tile)
                h0 = h0_pool.tile([B_LOC, D_H], F32R)
                nc.sync.dma_start(h0[:], H0[:, :].bitcast(F32R))
                hT = [None] * KH
                def pack_transposes(hseg_pair_src, c0, dest):
                    """Transpose two [B_LOC,128] column blocks of hseg_pair_src
                    into one [128, 2*B_LOC] psum tile; single cast to an f32r
                    hT tile holding chunks c0, c0+1."""
                    tps = scan_tps.tile([128, 2 * B_LOC], F32R)
                    for j in range(2):
                        nc.tensor.transpose(
                            tps[:, bass.ts(j, B_LOC)],
                            hseg_pair_src[:, 128 * j : 128 * (j + 1)],
                            identr[:B_LOC, :B_LOC],
                        )
                    ht2 = hT_pool.tile([128, 2 * B_LOC], F32R)
                    nc.vector.tensor_copy(ht2[:], tps[:])
                    dest[c0] = ht2[:, 0:B_LOC]
                    dest[c0 + 1] = ht2[:, B_LOC : 2 * B_LOC]


                for cpos in range(0, KH, 2):
                    pack_transposes(h0[:, 256 * (cpos // 2) :], cpos, hT)

                pending = None  # (hseg_q2, hseg_q3, dest) from previous step
                for t in range(t_steps):
                    wxb = wxb_pool.tile([B_LOC, D_H], F32)
                    nc.sync.dma_start(wxb[:], WXB[B_LOC * t : B_LOC * (t + 1), :])
                    hT_new = [None] * KH
                    hsegs = []
                    for q in range(4):
                        qsl = bass.ts(q, 256)
                        ps = scan_ps.tile([B_LOC, 256], F32)
                        for ki in range(KH):
                            if ki == 4 and pending is not None:
                                # previous step's chunks 4-7 (tanh completed
                                # during this step's q0 k0-k3 matmuls)
                                pack_transposes(pending[0], 4, pending[2])
                                pack_transposes(pending[1], 6, pending[2])
                                pending = None
                            nc.tensor.matmul(
                                ps[:],
                                hT[ki][:],
                                wh_sb[:, ki, qsl],
                                start=(ki == 0),
                                stop=(ki == KH - 1),
                            )
                        if q == 3 and t < t_steps - 1:
                            # chunks 0-3: q0/q1 tanh completed during q1-q3 MMs
                            pack_transposes(hsegs[0], 0, hT_new)
                            pack_transposes(hsegs[1], 2, hT_new)
                        nc.vector.tensor_add(ps[:], ps[:], wxb[:, qsl])
                        hseg = hseg_pool.tile([B_LOC, 256], F32R)
                        nc.scalar.activation(hseg[:], ps[:], Tanh)
                        if t == t_steps - 1:
                            nc.sync.dma_start(OUT[:, qsl].bitcast(F32R), hseg[:])
                        else:
                            hsegs.append(hseg)
                    if t < t_steps - 1:
                        pending = (hsegs[2], hsegs[3], hT_new)
                        hT = hT_new

    nc.compile()
    return nc


def _get(t_steps: int = T):
    if t_steps not in _BUILT:
        _BUILT[t_steps] = build(t_steps)
    return _BUILT[t_steps]


def run(inputs: dict, t_steps: int = T, trace: bool = False, **kw):
    """Shard inputs, run on 8 cores, gather. Returns (out, BassKernelResults)."""
    nc = _get(t_steps)
    X = np.ascontiguousarray(np.asarray(inputs["X"], dtype=np.float32))
    h = np.ascontiguousarray(np.asarray(inputs["h"], dtype=np.float32))
    W_enc = np.ascontiguousarray(np.asarray(inputs["W_enc"], dtype=np.float32))
    b_enc = np.ascontiguousarray(np.asarray(inputs["b_enc"], dtype=np.float32))
    W_h = np.ascontiguousarray(np.asarray(inputs["W_h"], dtype=np.float32))
    b_h = np.ascontiguousarray(np.asarray(inputs["b_h"], dtype=np.float32))
    assert X.shape == (t_steps, B, D_IN), X.shape

    in_maps = []
    for j in range(N_CORES):
        bs = slice(j * B_LOC, (j + 1) * B_LOC)
        in_maps.append(
            {
                "X": np.ascontiguousarray(X[:, bs, :]),
                "h": np.ascontiguousarray(h[bs, :]),
                "W_enc": W_enc,
                "b_enc": b_enc,
                "W_h": W_h,
                "b_h": b_h,
            }
        )
    res = run_bass_kernel_spmd(
        nc, in_maps, core_ids=list(range(N_CORES)), trace=trace, **kw
    )
    out = np.concatenate([r["out"] for r in res.results], axis=0)
    return out, res


def kernel(**inputs) -> np.ndarray:
    out, _ = run(inputs, T)
    return out


# revision 27
# speedup vs baseline: 1.0107x; 1.0067x over previous
"""Trainium2 Bass kernel for RNN classification forward pass.

Math (per reference):
    Wx = einsum('tbi,ih->tbh', X, W_enc) + b_enc          # encoder GEMM
    h_t = tanh(Wx_t + h_{t-1} @ W_h + b_h)                # T sequential steps
    return h_T                                            # [B, D_H]

Strategy: pure data-parallel over batch (8 cores x B_loc=16), no cross-core
communication.  Per-step collectives have a ~5-10us floor on trn2, which is
a non-starter for 512 sequential steps; and since a matmul's PE time is set
by the moving-operand stream (K/128 x N cycles, independent of M), the scan
costs the same ~8192 cyc/step per core no matter how the batch is sharded --
so data-parallel is as good as any partitioning that avoids communication.

Per core:
  Phase 1 (encoder): WxB[t,b,:] = X_shard @ W_enc + (b_enc + b_h) as one
    large GEMM over T*B_loc rows (full-128 stationary M), with PE-transposed
    X tiles as the stationary operand and fp32r matmuls (1 cyc/row at
    N>=256; plain fp32 would be 4 cyc/row).  The bias row is replicated to
    all 128 partitions once via a one-hot-stationary matmul and added in the
    DVE epilogue.  Result staged to a DRAM scratch buffer.
  Phase 2 (scan): state kept transposed (hT: 8 chunks of [128, 16]) so each
    step's matmuls use hT as the *stationary* operand (tiny 16-col weight
    loads) and stream W_h columns: psum[16,256] += hT_k.T @ W_h[k, qslice],
    4 psum quarters per step.  DVE adds WxB_t in place on PSUM, ACT applies
    tanh to SBUF, and PE transpose-mode ops (single-pass in fp32r) rebuild
    the hT chunks, two chunks packed per [128,32] psum tile with one cast.
    Transposes are scheduled where their semaphore waits are pre-satisfied:
    chunks 0-3 after this step's q3 matmuls, chunks 4-7 deferred into the
    NEXT step's q0 k-loop (at k=4), hiding the add+tanh latency behind
    matmul streams.

Measured on trn2 (8 cores): ~2.72 ms HW exec, rel err ~2.2e-4 vs fp32 ref.
"""
import numpy as np

import concourse.bass as bass
import concourse.mybir as mybir
import concourse.tile as tile
from concourse import bacc
from concourse.bass_utils import run_bass_kernel_spmd
from concourse.masks import make_identity

T, B, D_IN, D_H = 512, 128, 512, 1024
N_CORES = 8
B_LOC = B // N_CORES  # 16

F32 = mybir.dt.float32
F32R = mybir.dt.float32r
Tanh = mybir.ActivationFunctionType.Tanh

_BUILT = {}


def build(t_steps: int = T):
    """Build the per-core Bass module (SPMD; same program on all cores)."""
    nc = bacc.Bacc("TRN2", target_bir_lowering=False, debug=False)

    X = nc.dram_tensor("X", [t_steps, B_LOC, D_IN], F32, kind="ExternalInput").ap()
    H0 = nc.dram_tensor("h", [B_LOC, D_H], F32, kind="ExternalInput").ap()
    WENC = nc.dram_tensor("W_enc", [D_IN, D_H], F32, kind="ExternalInput").ap()
    BENC = nc.dram_tensor("b_enc", [D_H], F32, kind="ExternalInput").ap()
    WH = nc.dram_tensor("W_h", [D_H, D_H], F32, kind="ExternalInput").ap()
    BH = nc.dram_tensor("b_h", [D_H], F32, kind="ExternalInput").ap()
    OUT = nc.dram_tensor("out", [B_LOC, D_H], F32, kind="ExternalOutput").ap()
    WXB = nc.dram_tensor("wxb_scratch", [t_steps * B_LOC, D_H], F32).ap()

    KI = D_IN // 128  # 4 K-chunks for the encoder contraction
    KH = D_H // 128   # 8 K-chunks for the recurrent contraction
    NSEG = D_H // 256  # 4 scan output segments (psum tiles of [16, 256])
    ROWS = t_steps * B_LOC
    assert ROWS % 128 == 0
    NCH = ROWS // 128  # (t,b)-row chunks for the encoder

    with tile.TileContext(nc) as tc:
        with (
            tc.tile_pool(name="const", bufs=1) as const_pool,
        ):
            # ---- constants ----
            ident = const_pool.tile([128, 128], F32)
            make_identity(nc, ident[:])
            identr = const_pool.tile([128, 128], F32R)
            nc.vector.tensor_copy(identr[:], ident[:])

            # e0: ones in partitions 0-1 -> bias matmul adds bvec row0+row1.
            e0f = const_pool.tile([128, 128], F32)
            nc.gpsimd.memset(e0f[:], 0.0)
            nc.gpsimd.memset(e0f[0:2, :], 1.0)
            e0 = const_pool.tile([128, 128], F32R)
            nc.vector.tensor_copy(e0[:], e0f[:])

            # bvec: row 0 = b_enc, row 1 = b_h, other partitions zero.
            zeros_f = const_pool.tile([128, D_H], F32)
            nc.gpsimd.memset(zeros_f[:], 0.0)
            bvec = const_pool.tile([128, D_H], F32R)
            nc.vector.tensor_copy(bvec[:], zeros_f[:])
            nc.sync.dma_start(bvec[0:1, :], BENC[None, :].bitcast(F32R))
            nc.sync.dma_start(bvec[1:2, :], BH[None, :].bitcast(F32R))

            # resident weights, K-chunked with K on partitions
            wh_sb = const_pool.tile([128, KH, D_H], F32R)
            nc.sync.dma_start(
                wh_sb[:], WH.rearrange("(ko ki) n -> ki ko n", ki=128).bitcast(F32R)
            )
            wenc_sb = const_pool.tile([128, KI, D_H], F32R)
            nc.sync.dma_start(
                wenc_sb[:], WENC.rearrange("(ko ki) n -> ki ko n", ki=128).bitcast(F32R)
            )

            # bias_rep: every row = b_enc + b_h (one e0-matmul per 512 cols)
            bias_rep = const_pool.tile([128, D_H], F32)
            with tc.tile_pool(name="bias_ps", bufs=2, space="PSUM") as bias_ps:
                for n in range(D_H // 512):
                    nsl = bass.ts(n, 512)
                    bps = bias_ps.tile([128, 512], F32)
                    nc.tensor.matmul(bps[:], e0[:], bvec[:, nsl], start=True, stop=True)
                    nc.scalar.copy(bias_rep[:, nsl], bps[:])

            # ---- phase 1: encoder GEMM -> WXB scratch ----
            Xf = X.rearrange("t b i -> (t b) i")
            with (
                tc.tile_pool(name="enc_x", bufs=6) as x_pool,
                tc.tile_pool(name="enc_xt", bufs=4) as xt_pool,
                tc.tile_pool(name="enc_ps", bufs=3, space="PSUM") as enc_ps,
                tc.tile_pool(name="enc_tps", bufs=3, space="PSUM") as enc_tps,
                tc.tile_pool(name="enc_out", bufs=4) as enc_out,
            ):
                for c in range(NCH):
                    xnat = x_pool.tile([128, D_IN], F32R)
                    nc.sync.dma_start(
                        xnat[:], Xf[128 * c : 128 * (c + 1), :].bitcast(F32R)
                    )
                    xT = xt_pool.tile([128, KI, 128], F32R)
                    for k in range(KI):
                        tps = enc_tps.tile([128, 128], F32R)
                        nc.tensor.transpose(
                            tps[:], xnat[:, 128 * k : 128 * (k + 1)], identr[:]
                        )
                        nc.scalar.copy(xT[:, k, :], tps[:])
                    for n in range(D_H // 512):
                        nsl = bass.ts(n, 512)
                        ps = enc_ps.tile([128, 512], F32)
                        for k in range(KI):
                            nc.tensor.matmul(
                                ps[:],
                                xT[:, k, :],
                                wenc_sb[:, k, nsl],
                                start=(k == 0),
                                stop=(k == KI - 1),
                            )
                        wxbo = enc_out.tile([128, 512], F32)
                        nc.vector.tensor_add(wxbo[:], ps[:], bias_rep[:, nsl])
                        nc.sync.dma_start(WXB[128 * c : 128 * (c + 1), nsl], wxbo[:])

            # ---- phase 2: sequential scan ----
            with (
                tc.tile_pool(name="wxb", bufs=6) as wxb_pool,
                tc.tile_pool(name="hT", bufs=24) as hT_pool,
                tc.tile_pool(name="hseg", bufs=8) as hseg_pool,
                tc.tile_pool(name="scan_ps", bufs=5, space="PSUM") as scan_ps,
                tc.tile_pool(name="scan_tps", bufs=3, space="PSUM") as scan_tps,
                tc.tile_pool(name="h0p", bufs=1) as h0_pool,
            ):
                # initial state -> transposed chunks (packed 2 per tile)
                h0 = h0_pool.tile([B_LOC, D_H], F32R)
                nc.sync.dma_start(h0[:], H0[:, :].bitcast(F32R))
                hT = [None] * KH
                def pack_transposes(hseg_pair_src, c0, dest):
                    """Transpose two [B_LOC,128] column blocks of hseg_pair_src
                    into one [128, 2*B_LOC] psum tile; single cast to an f32r
                    hT tile holding chunks c0, c0+1."""
                    tps = scan_tps.tile([128, 2 * B_LOC], F32R)
                    for j in range(2):
                        nc.tensor.transpose(
                            tps[:, bass.ts(j, B_LOC)],
                            hseg_pair_src[:, 128 * j : 128 * (j + 1)],
                            identr[:B_LOC, :B_LOC],
                        )
                    ht2 = hT_pool.tile([128, 2 * B_LOC], F32R)
                    nc.vector.tensor_copy(ht2[:], tps[:])
                    dest[c0] = ht2[:, 0:B_LOC]
                    dest[c0 + 1] = ht2[:, B_LOC : 2 * B_LOC]


                for cpos in range(0, KH, 2):
                    pack_transposes(h0[:, 256 * (cpos // 2) :], cpos, hT)

                pending = None  # (hseg_q2, hseg_q3, dest) from previous step
                for t in range(t_steps):
                    wxb = wxb_pool.tile([B_LOC, D_H], F32)
                    nc.sync.dma_start(wxb[:], WXB[B_LOC * t : B_LOC * (t + 1), :])
                    hT_new = [None] * KH
                    hsegs = []
                    for q in range(4):
                        qsl = bass.ts(q, 256)
                        ps = scan_ps.tile([B_LOC, 256], F32)
                        for ki in range(KH):
                            if ki == 4 and pending is not None:
                                # previous step's chunks 4-7 (tanh completed
                                # during this step's q0 k0-k3 matmuls)
                                pack_transposes(pending[0], 4, pending[2])
                                pack_transposes(pending[1], 6, pending[2])
                                pending = None
                            nc.tensor.matmul(
                                ps[:],
                                hT[ki][:],
                                wh_sb[:, ki, qsl],
                                start=(ki == 0),
                                stop=(ki == KH - 1),
                            )
                        if q == 3 and t < t_steps - 1:
                            # chunks 0-3: q0/q1 tanh completed during q1-q3 MMs
                            pack_transposes(hsegs[0], 0, hT_new)
                            pack_transposes(hsegs[1], 2, hT_new)
                        nc.vector.tensor_add(ps[:], ps[:], wxb[:, qsl])
                        hseg = hseg_pool.tile([B_LOC, 256], F32R)
                        nc.scalar.activation(hseg[:], ps[:], Tanh)
                        if t == t_steps - 1:
                            nc.sync.dma_start(OUT[:, qsl].bitcast(F32R), hseg[:])
                        else:
                            hsegs.append(hseg)
                    if t < t_steps - 1:
                        pending = (hsegs[2], hsegs[3], hT_new)
                        hT = hT_new

    nc.compile()
    return nc


def _get(t_steps: int = T):
    if t_steps not in _BUILT:
        _BUILT[t_steps] = build(t_steps)
    return _BUILT[t_steps]


def run(inputs: dict, t_steps: int = T, trace: bool = False, **kw):
    """Shard inputs, run on 8 cores, gather. Returns (out, BassKernelResults)."""
    nc = _get(t_steps)
    X = np.ascontiguousarray(np.asarray(inputs["X"], dtype=np.float32))
    h = np.ascontiguousarray(np.asarray(inputs["h"], dtype=np.float32))
    W_enc = np.ascontiguousarray(np.asarray(inputs["W_enc"], dtype=np.float32))
    b_enc = np.ascontiguousarray(np.asarray(inputs["b_enc"], dtype=np.float32))
    W_h = np.ascontiguousarray(np.asarray(inputs["W_h"], dtype=np.float32))
    b_h = np.ascontiguousarray(np.asarray(inputs["b_h"], dtype=np.float32))
    assert X.shape == (t_steps, B, D_IN), X.shape

    in_maps = []
    for j in range(N_CORES):
        bs = slice(j * B_LOC, (j + 1) * B_LOC)
        in_maps.append(
            {
                "X": np.ascontiguousarray(X[:, bs, :]),
                "h": np.ascontiguousarray(h[bs, :]),
                "W_enc": W_enc,
                "b_enc": b_enc,
                "W_h": W_h,
                "b_h": b_h,
            }
        )
    res = run_bass_kernel_spmd(
        nc, in_maps, core_ids=list(range(N_CORES)), trace=trace, **kw
    )
    out = np.concatenate([r["out"] for r in res.results], axis=0)
    return out, res


def kernel(**inputs) -> np.ndarray:
    out, _ = run(inputs, T)
    return out
